# revision 1
# baseline (speedup 1.0000x reference)
"""Trainium2 Bass kernel for nn_CtoX (gnn_message_passing).

Computes, per batch b:
  stage1 (CtoE): block-pair stats (mean/min/max/std with pairwise masks) of
     delta1[b] over 16x16 atom blocks -> z[b, 16, 16, 256] -> E = z @ W1.T + b1
  stage2 (EtoX): masked stats of E over its second block axis -> zE[b,16,256]
     -> out = zE @ W2.T + b2   (out: [4, 16, 256])

Sharding: 8 cores = (4 batches) x (2 halves of the first nm axis).
Each core handles delta1[b, ih*128:(ih+1)*128, :, :] (8 MiB) and produces
out[b, ih*8:(ih+1)*8, :] with zero cross-core communication.

Per-core program layout (partitions = i, the first atom axis):
  - min/max: biased = d + BIG*(1-cm1[i]*cm2[j]) via one tensor_tensor add per
    J-slice against a precomputed [128, 256] bias field (broadcast over c),
    then a grouped free-dim reduce over jj.  Exact reference arithmetic.
  - sums S1 = sum(d), Sm = sum(mask*d), S2 = sum(mask*d^2): per-j matmuls with
    lhsT columns [I-block indicator | cm1*cm2[j]*indicator], PSUM-accumulated
    over jj (so the j-contraction happens in PSUM).  d^2 from ScalarE Square.
  - second stage: PE transposes (i -> partitions c) + small reduces/arithmetic
    build the z matrix directly in [feature, row] layout so the linear layers
    run as plain matmuls with host-transposed weights.
"""

import numpy as np
from contextlib import ExitStack

BIG = 100000.0
EPS = 1e-8

BS, NM, DC, DX = 4, 256, 256, 64  # note: delta1 is [bs, nm, nm, dc=64]; dx=256
# actual dims
D_C = 64      # channel dim of delta1
D_X = 256     # output feature dim
MA = 16       # atoms per block
NBLK = 16     # number of blocks along each nm axis
P = 128       # partitions per core (half of nm)
NI = 8        # I-blocks per core
NJ = 16       # J-blocks

# cpack column offsets (one packed [128, CPACK_COLS] constants tensor)
OFF_CM1 = 0
OFF_CM2 = 1
OFF_EM = OFF_CM2 + 256
OFF_IDENT = OFF_EM + 16
OFF_BIND = OFF_IDENT + 128
OFF_W1T = OFF_BIND + 8
OFF_W2T = OFF_W1T + 256
OFF_B1 = OFF_W2T + 512
OFF_B2 = OFF_B1 + 1
CPACK_COLS = OFF_B2 + 2

_CACHE = {}


def _build_program():
    import concourse.bass as bass
    import concourse.bacc as bacc
    import concourse.tile as tile
    import concourse.mybir as mybir

    f32 = mybir.dt.float32
    Alu = mybir.AluOpType
    Act = mybir.ActivationFunctionType
    AX = mybir.AxisListType

    nc = bacc.Bacc()

    d_in = nc.dram_tensor("d", [P, NM, D_C], f32, kind="ExternalInput")
    cpack_in = nc.dram_tensor("cpack", [P, CPACK_COLS], f32, kind="ExternalInput")
    out_t = nc.dram_tensor("out_t", [D_X, NI], f32, kind="ExternalOutput")

    with tile.TileContext(nc) as tc, ExitStack() as ctx:
        consts = ctx.enter_context(tc.tile_pool(name="consts", bufs=1))
        small = ctx.enter_context(tc.tile_pool(name="small", bufs=1))

        # ---------- constant loads: ONE DMA for everything small ----------
        cpak = consts.tile([P, CPACK_COLS], f32)
        nc.sync.dma_start(out=cpak, in_=cpack_in[:])
        cm1s = cpak[:, OFF_CM1 : OFF_CM1 + 1]
        cm2rep = cpak[:, OFF_CM2 : OFF_CM2 + NM]
        emrep = cpak[:, OFF_EM : OFF_EM + NJ]
        ident = cpak[:, OFF_IDENT : OFF_IDENT + P]
        bind = cpak[:, OFF_BIND : OFF_BIND + NI]
        w1t_a = cpak[:, OFF_W1T : OFF_W1T + 128]
        w1t_b = cpak[:, OFF_W1T + 128 : OFF_W1T + 256]
        w2t_a = cpak[:, OFF_W2T : OFF_W2T + 256]
        w2t_b = cpak[:, OFF_W2T + 256 : OFF_W2T + 512]
        b1c = cpak[:, OFF_B1 : OFF_B1 + 1]
        b2c_a = cpak[:, OFF_B2 : OFF_B2 + 1]
        b2c_b = cpak[:, OFF_B2 + 1 : OFF_B2 + 2]

        # ---------- mask-derived fields ----------
        T2 = consts.tile([P, NM], f32)  # cm1[i]*cm2[j]
        nc.scalar.mul(T2[:], cm2rep, cm1s)
        biasF = consts.tile([P, NM], f32)  # BIG*(1 - cm1*cm2)
        nc.vector.tensor_scalar(biasF, T2[:], -BIG, BIG, Alu.mult, Alu.add)
        biasFn = consts.tile([P, NM], f32)  # -biasF
        nc.vector.tensor_scalar(biasFn, T2[:], BIG, -BIG, Alu.mult, Alu.add)

        # block indicator bind[i, I] = 1 if i//16 == I (from cpack)
        bindw = consts.tile([P, NI], f32)  # cm1-weighted indicator
        nc.scalar.mul(bindw[:], bind, cm1s)

        # lhsT field for the sums matmuls: [128, j, 16]
        #   cols 0-7 : bind (plain)        cols 8-15 : bind*cm1*cm2[j]
        lhsTs = consts.tile([P, NM, 16], f32)
        nc.vector.tensor_copy(
            out=lhsTs[:, :, 0:8],
            in_=bind.unsqueeze(1).broadcast_to([P, NM, NI]),
        )
        nc.vector.tensor_tensor(
            out=lhsTs[:, :, 8:16],
            in0=bindw[:].unsqueeze(1).broadcast_to([P, NM, NI]),
            in1=cm2rep.unsqueeze(2).broadcast_to([P, NM, NI]),
            op=Alu.mult,
        )

        # counts / divide / std-factors, replicated on all 128 partitions:
        #   cnt1rep[p, I] = sum_a cm1[I*16+a]   (via PE, lhsT = cm1 bcast)
        psum_misc = ctx.enter_context(
            tc.tile_pool(name="psum_misc", bufs=1, space="PSUM")
        )
        cnt1_ps = psum_misc.tile([P, NI], f32)
        nc.tensor.matmul(
            cnt1_ps[:],
            lhsT=cm1s.broadcast_to([P, P]),
            rhs=bind,
            start=True,
            stop=True,
        )
        cnt1rep = consts.tile([P, NI], f32)
        nc.scalar.copy(cnt1rep[:], cnt1_ps[:])
        cnt2rep = consts.tile([P, NJ], f32)  # sum of cm2 within each J block
        nc.vector.tensor_reduce(
            out=cnt2rep[:],
            in_=cm2rep.rearrange("p (J a) -> p J a", a=MA),
            axis=AX.X,
            op=Alu.add,
        )
        divR = consts.tile([P, NI, NJ], f32)
        nc.vector.tensor_tensor(
            out=divR[:],
            in0=cnt1rep[:].unsqueeze(2).broadcast_to([P, NI, NJ]),
            in1=cnt2rep[:].unsqueeze(1).broadcast_to([P, NI, NJ]),
            op=Alu.mult,
        )
        nc.vector.tensor_scalar_add(divR[:], divR[:], EPS)
        recipD = consts.tile([P, NI, NJ], f32)
        nc.vector.reciprocal(recipD[:], divR[:])
        nfac = consts.tile([P, NI, NJ], f32)  # 1 - EPS/div
        nc.vector.tensor_scalar(nfac[:], recipD[:], -EPS, 1.0, Alu.mult, Alu.add)

        # ---------- big J-loop ----------
        # umm[:, J, 0:64] = per-(i, J, c) biased max; [:, J, 64:128] = biased
        # min -- packed so ONE transpose per J lands ma-feats at partitions
        # 0:64 and mi-feats at 64:128.
        umm = consts.tile([P, NJ, P], f32)

        with tc.tile_pool(name="psum_sums", bufs=1, space="PSUM") as psum_sums, \
             tc.tile_pool(name="loop", bufs=3) as loop_pool, \
             tc.tile_pool(name="btmp", bufs=3) as btmp_pool:
            S1m_ps = psum_sums.tile([16, NJ, D_C], f32)
            S2_ps = psum_sums.tile([8, NJ, D_C], f32)

            for J in range(NJ):
                dt = loop_pool.tile([P, MA, D_C], f32, tag="dt")
                nc.sync.dma_start(out=dt, in_=d_in[:, J * MA : (J + 1) * MA, :])

                bt = btmp_pool.tile([P, MA, D_C], f32, tag="bt")
                nc.vector.tensor_tensor(
                    out=bt,
                    in0=dt[:],
                    in1=biasF[:, J * MA : (J + 1) * MA]
                    .unsqueeze(2)
                    .broadcast_to([P, MA, D_C]),
                    op=Alu.add,
                )
                nc.vector.tensor_reduce(
                    out=umm[:, J, 64:128],
                    in_=bt[:].rearrange("p a c -> p c a"),
                    axis=AX.X,
                    op=Alu.min,
                )
                bt2 = btmp_pool.tile([P, MA, D_C], f32, tag="bt")
                nc.vector.tensor_tensor(
                    out=bt2,
                    in0=dt[:],
                    in1=biasFn[:, J * MA : (J + 1) * MA]
                    .unsqueeze(2)
                    .broadcast_to([P, MA, D_C]),
                    op=Alu.add,
                )
                nc.vector.tensor_reduce(
                    out=umm[:, J, 0:64],
                    in_=bt2[:].rearrange("p a c -> p c a"),
                    axis=AX.X,
                    op=Alu.max,
                )

                sq = loop_pool.tile([P, MA, D_C], f32, tag="sq")
                nc.scalar.activation(out=sq, in_=dt[:], func=Act.Square)

                for jj in range(MA):
                    j = J * MA + jj
                    nc.tensor.matmul(
                        S1m_ps[:, J, :],
                        lhsT=lhsTs[:, j, 0:16],
                        rhs=dt[:, jj, :],
                        start=(jj == 0),
                        stop=(jj == MA - 1),
                    )
                    nc.tensor.matmul(
                        S2_ps[:, J, :],
                        lhsT=lhsTs[:, j, 8:16],
                        rhs=sq[:, jj, :],
                        start=(jj == 0),
                        stop=(jj == MA - 1),
                    )

            # evacuate sums (16/8-partition tiles), duplicated into both
            # 64-column halves so one transpose serves lo and hi features.
            Ssb = small.tile([16, NJ, P], f32)
            nc.scalar.copy(Ssb[:, :, 0:64], S1m_ps[:])
            nc.scalar.copy(Ssb[:, :, 64:128], S1m_ps[:])
            S2sb = small.tile([8, NJ, P], f32)
            nc.scalar.copy(S2sb[:, :, 0:64], S2_ps[:])
            nc.scalar.copy(S2sb[:, :, 64:128], S2_ps[:])

        # ---------- stage 2: transposes to [feature, row] layout ----------
        # rows r = I*16 + J; feature chunks:
        #   rhs_z0: [0:64] m-feats, [64:128] mi-feats
        #   rhs_z1: [0:64] ma-feats, [64:128] std-feats
        rhs_z0 = small.tile([P, P], f32)
        rhs_z1 = small.tile([P, P], f32)

        with tc.tile_pool(name="psum_t", bufs=2, space="PSUM") as psum_t, \
             tc.tile_pool(name="psum_ts", bufs=1, space="PSUM") as psum_ts:
            # min/max: one transpose per J: [128 i, 128(ma|mi)] ->
            # [128(ma@lo|mi@hi), 128 i], then reduce over a within I-blocks.
            for J in range(NJ):
                tp = psum_t.tile([P, P], f32, tag="tp")
                nc.tensor.transpose(
                    out=tp[:], in_=umm[:, J, :], identity=ident
                )
                nc.vector.tensor_reduce(
                    out=rhs_z0[64:128, :]
                    .rearrange("p (I J) -> p I J", J=NJ)[:, :, J],
                    in_=tp[64:128, :].rearrange("p (I a) -> p I a", a=MA),
                    axis=AX.X,
                    op=Alu.min,
                )
                nc.vector.tensor_reduce(
                    out=rhs_z1[0:64, :]
                    .rearrange("p (I J) -> p I J", J=NJ)[:, :, J],
                    in_=tp[0:64, :].rearrange("p (I a) -> p I a", a=MA),
                    axis=AX.X,
                    op=Alu.max,
                )

            # sums: one transpose per J: [16, 128(dup)] -> [128, 16]
            # (columns = [S1T | SmT]); rows 0:64 serve m, 64:128 serve std.
            SST_ps = psum_ts.tile([P, 16, NJ], f32, tag="sst")
            S2T_ps = psum_ts.tile([P, NI, NJ], f32, tag="s2t")
            for J in range(NJ):
                nc.tensor.transpose(
                    out=SST_ps[:, :, J],
                    in_=Ssb[0:16, J, :],
                    identity=ident[0:16, 0:16],
                )
                nc.tensor.transpose(
                    out=S2T_ps[:, :, J],
                    in_=S2sb[0:8, J, :],
                    identity=ident[0:8, 0:8],
                )

            SST = small.tile([P, 16, NJ], f32)
            nc.scalar.copy(SST[:], SST_ps[:])
            S2T = small.tile([P, NI, NJ], f32)
            nc.scalar.copy(S2T[64:128], S2T_ps[64:128])
            S1T = SST[:, 0:8, :]
            SmT = SST[:, 8:16, :]

        # m = S1/div  (lo half -> m-features; hi half feeds std)
        mT = small.tile([P, NI, NJ], f32)
        nc.vector.tensor_tensor(out=mT[:], in0=S1T, in1=recipD[:], op=Alu.mult)
        nc.vector.tensor_copy(
            out=rhs_z0[0:64, :].rearrange("p (I J) -> p I J", J=NJ),
            in_=mT[0:64],
        )
        # std = S2/div - 2*m*(Sm/div) + m^2*nfac     (hi half only)
        A = small.tile([P, NI, NJ], f32)
        nc.vector.tensor_tensor(
            out=A[64:128], in0=S2T[64:128], in1=recipD[64:128], op=Alu.mult
        )
        Bq = small.tile([P, NI, NJ], f32)
        nc.vector.tensor_tensor(
            out=Bq[64:128], in0=SmT[64:128], in1=recipD[64:128], op=Alu.mult
        )
        nc.vector.tensor_tensor(
            out=Bq[64:128], in0=Bq[64:128], in1=mT[64:128], op=Alu.mult
        )
        nc.vector.tensor_scalar(
            Bq[64:128], Bq[64:128], -2.0, None, Alu.mult
        )  # -2*m*Sm/div
        nc.vector.tensor_tensor(
            out=A[64:128], in0=A[64:128], in1=Bq[64:128], op=Alu.add
        )
        Cq = small.tile([P, NI, NJ], f32)
        nc.vector.tensor_tensor(
            out=Cq[64:128], in0=mT[64:128], in1=mT[64:128], op=Alu.mult
        )
        nc.vector.tensor_tensor(
            out=Cq[64:128], in0=Cq[64:128], in1=nfac[64:128], op=Alu.mult
        )
        nc.vector.tensor_tensor(
            out=rhs_z1[64:128, :].rearrange("p (I J) -> p I J", J=NJ),
            in0=A[64:128],
            in1=Cq[64:128],
            op=Alu.add,
        )

        # ---------- E = z @ W1.T + b1 (duplicated channels on 128 parts) ----
        with tc.tile_pool(name="psum_e", bufs=1, space="PSUM") as psum_e:
            E_ps = psum_e.tile([P, P], f32)
            nc.tensor.matmul(
                E_ps[:], lhsT=w1t_a, rhs=rhs_z0[:], start=True, stop=False
            )
            nc.tensor.matmul(
                E_ps[:], lhsT=w1t_b, rhs=rhs_z1[:], start=False, stop=True
            )
            E_T = small.tile([P, P], f32)  # [128(dup chan), 128 rows=(I,J)]
            nc.scalar.activation(
                out=E_T[:], in_=E_ps[:], func=Act.Identity, bias=b1c, scale=1.0
            )

        # ---------- stage 2 of the net: masked stats over J ----------
        cntE = small.tile([P, 1], f32)
        nc.vector.tensor_reduce(out=cntE[:], in_=emrep, axis=AX.X, op=Alu.add)
        recipE = small.tile([P, 1], f32)
        nc.vector.reciprocal(recipE[:], cntE[:])
        biasE = small.tile([P, NJ], f32)  # BIG*(1-em)
        nc.vector.tensor_scalar(biasE[:], emrep, -BIG, BIG, Alu.mult, Alu.add)
        biasEn = small.tile([P, NJ], f32)
        nc.vector.tensor_scalar(biasEn[:], emrep, BIG, -BIG, Alu.mult, Alu.add)

        E_r = E_T[:].rearrange("p (I J) -> p I J", J=NJ)
        zE0 = small.tile([P, NI], f32)  # [0:64] mE, [64:128] miE
        zE1 = small.tile([P, NI], f32)  # [0:64] maE, [64:128] stdE

        # mE (all partitions; lo half is the m-feature, hi half feeds stdE)
        mE = small.tile([P, NI], f32)
        nc.vector.tensor_reduce(out=mE[:], in_=E_r, axis=AX.X, op=Alu.add)
        nc.scalar.mul(mE[:], mE[:], recipE[:])
        nc.vector.tensor_copy(out=zE0[0:64, :], in_=mE[0:64, :])

        # miE on hi half
        bE = small.tile([P, NI, NJ], f32)
        nc.vector.tensor_tensor(
            out=bE[64:128],
            in0=E_r[64:128],
            in1=biasE[64:128].unsqueeze(1).broadcast_to([64, NI, NJ]),
            op=Alu.add,
        )
        nc.vector.tensor_reduce(
            out=zE0[64:128, :], in_=bE[64:128], axis=AX.X, op=Alu.min
        )
        # maE on lo half
        nc.vector.tensor_tensor(
            out=bE[0:64],
            in0=E_r[0:64],
            in1=biasEn[0:64].unsqueeze(1).broadcast_to([64, NI, NJ]),
            op=Alu.add,
        )
        nc.vector.tensor_reduce(
            out=zE1[0:64, :], in_=bE[0:64], axis=AX.X, op=Alu.max
        )
        # stdE on hi half: sum(em*(E-mE)^2)/denom
        dev = small.tile([P, NI, NJ], f32)
        nc.vector.tensor_tensor(
            out=dev[64:128],
            in0=E_r[64:128],
            in1=mE[64:128].unsqueeze(2).broadcast_to([64, NI, NJ]),
            op=Alu.subtract,
        )
        nc.vector.tensor_tensor(
            out=dev[64:128], in0=dev[64:128], in1=dev[64:128], op=Alu.mult
        )
        nc.vector.tensor_tensor(
            out=dev[64:128],
            in0=dev[64:128],
            in1=emrep[64:128].unsqueeze(1).broadcast_to([64, NI, NJ]),
            op=Alu.mult,
        )
        nc.vector.tensor_reduce(
            out=zE1[64:128, :], in_=dev[64:128], axis=AX.X, op=Alu.add
        )
        nc.scalar.mul(zE1[64:128, :], zE1[64:128, :], recipE[64:128, :])

        # ---------- out = zE @ W2.T + b2 ----------
        with tc.tile_pool(name="psum_o", bufs=1, space="PSUM") as psum_o:
            outa_ps = psum_o.tile([128, NI], f32)
            outb_ps = psum_o.tile([128, NI], f32)
            nc.tensor.matmul(
                outa_ps[:], lhsT=w2t_a[:, 0:128], rhs=zE0[:], start=True, stop=False
            )
            nc.tensor.matmul(
                outa_ps[:], lhsT=w2t_b[:, 0:128], rhs=zE1[:], start=False, stop=True
            )
            nc.tensor.matmul(
                outb_ps[:], lhsT=w2t_a[:, 128:256], rhs=zE0[:], start=True, stop=False
            )
            nc.tensor.matmul(
                outb_ps[:], lhsT=w2t_b[:, 128:256], rhs=zE1[:], start=False,
                stop=True,
            )
            outa = small.tile([128, NI], f32)
            nc.scalar.activation(
                out=outa[:], in_=outa_ps[:], func=Act.Identity, bias=b2c_a,
                scale=1.0,
            )
            outb = small.tile([128, NI], f32)
            nc.scalar.activation(
                out=outb[:], in_=outb_ps[:], func=Act.Identity, bias=b2c_b,
                scale=1.0,
            )
            nc.sync.dma_start(out=out_t[0:128, :], in_=outa[:])
            nc.sync.dma_start(out=out_t[128:256, :], in_=outb[:])

    nc.finalize()  # Bacc: runs compile() (wait splitting, reg alloc, ...)
    return nc


def _get_program():
    if "nc" not in _CACHE:
        _CACHE["nc"] = _build_program()
    return _CACHE["nc"]


def _make_in_maps(delta1, c_mask1, c_mask2, e_mask2, W1, b1, W2, b2):
    delta1 = np.asarray(delta1, dtype=np.float32)
    c_mask1 = np.asarray(c_mask1, dtype=np.float32)
    c_mask2 = np.asarray(c_mask2, dtype=np.float32)
    e_mask2 = np.asarray(e_mask2, dtype=np.float32)
    W1 = np.asarray(W1, dtype=np.float32)
    b1 = np.asarray(b1, dtype=np.float32)
    W2 = np.asarray(W2, dtype=np.float32)
    b2 = np.asarray(b2, dtype=np.float32)

    w1t = np.concatenate([W1.T, W1.T], axis=1)  # [256, 128] (dup out-chan)
    w2t = W2.T  # [256, 256]
    bindm = np.zeros((128, 8), dtype=np.float32)
    for i in range(128):
        bindm[i, i // 16] = 1.0
    identm = np.eye(128, dtype=np.float32)

    def make_cpack(b):
        cp = np.zeros((128, CPACK_COLS), dtype=np.float32)
        cp[:, OFF_EM : OFF_EM + 16] = e_mask2[b, 0, :, 0][None, :]
        cp[:, OFF_IDENT : OFF_IDENT + 128] = identm
        cp[:, OFF_BIND : OFF_BIND + 8] = bindm
        cp[:, OFF_W1T : OFF_W1T + 128] = w1t[0:128, :]
        cp[:, OFF_W1T + 128 : OFF_W1T + 256] = w1t[128:256, :]
        cp[:, OFF_W2T : OFF_W2T + 256] = w2t[0:128, :]
        cp[:, OFF_W2T + 256 : OFF_W2T + 512] = w2t[128:256, :]
        cp[:, OFF_B1] = np.concatenate([b1, b1])
        cp[:, OFF_B2] = b2[0:128]
        cp[:, OFF_B2 + 1] = b2[128:256]
        cp[:, OFF_CM2 : OFF_CM2 + 256] = c_mask2[b, 0, :, 0][None, :]
        return cp

    in_maps = []
    for k in range(8):
        b, ih = k // 2, k % 2
        cp = make_cpack(b)
        cp[:, OFF_CM1] = c_mask1[b, ih * 128 : (ih + 1) * 128, 0, 0]
        in_maps.append(
            dict(
                d=np.ascontiguousarray(delta1[b, ih * 128 : (ih + 1) * 128]),
                cpack=cp,
            )
        )
    return in_maps


def _assemble(results):
    out = np.empty((4, 16, 256), dtype=np.float32)
    for k in range(8):
        b, ih = k // 2, k % 2
        out[b, ih * 8 : (ih + 1) * 8, :] = results[k]["out_t"].T
    return out


def run(trace=False, **inputs):
    from concourse.bass_utils import run_bass_kernel_spmd

    nc = _get_program()
    in_maps = _make_in_maps(**inputs)
    res = run_bass_kernel_spmd(
        nc, in_maps, core_ids=list(range(8)), trace=trace
    )
    return _assemble(res.results), res


def kernel(**inputs):
    out, _ = run(trace=False, **inputs)
    return out



# revision 11
# speedup vs baseline: 1.1251x; 1.1251x over previous
"""Trainium2 Bass kernel for nn_CtoX (gnn_message_passing).

Computes, per batch b:
  stage1 (CtoE): block-pair stats (mean/min/max/std with pairwise masks) of
     delta1[b] over 16x16 atom blocks -> z[b, 16, 16, 256] -> E = z @ W1.T + b1
  stage2 (EtoX): masked stats of E over its second block axis -> zE[b,16,256]
     -> out = zE @ W2.T + b2   (out: [4, 16, 256])

Sharding: 8 cores = (4 batches) x (2 halves of the first nm axis).
Each core handles delta1[b, ih*128:(ih+1)*128, :, :] (8 MiB) and produces
out[b, ih*8:(ih+1)*8, :] with zero cross-core communication.

Engine balance (v2):
  - All mask-derived constants (lhsT fields, bias fields, reciprocal
    divides) are precomputed on the HOST and shipped in one cpack DMA --
    no on-device setup passes.
  - J-loop min path (bias-add + grouped reduce) runs on Vector for all J;
    max path runs on GpSimd for J<GSPLIT and Vector for the rest
    (GpSimd runs DVE ops at ~0.42/0.60 efficiency, so it takes fewer J).
  - d and d^2 are packed side by side in one [128, 16, 128] tile so the
    three sums S1/Sm/S2 come from ONE matmul per j (lhsT cols [bind |
    bind*cm1*cm2[j]], rhs [d | d^2]) PSUM-accumulated over jj.
  - stage-2 min/max: per-J PE transposes into a PSUM ring, then batched
    4-J grouped reduces (amortizes the ~1us/instr DVE overhead).
"""

import numpy as np
from contextlib import ExitStack

BIG = 100000.0
EPS = 1e-8

D_C = 64      # channel dim of delta1
D_X = 256     # output feature dim
MA = 16       # atoms per block
NBLK = 16     # number of blocks along each nm axis
P = 128       # partitions per core (half of nm)
NI = 8        # I-blocks per core
NJ = 16       # J-blocks
NM = 256

GMIN = 6      # J < GMIN: min-path bias-add on GpSimd (else Vector).
              # Max-path bias-adds all run on GpSimd; reduces are
              # Vector-only (GpSimd can't do free-axis reduces).

# cpack column offsets (one packed [128, CPACK_COLS] constants tensor)
# -- section A (loop-critical, first DMA) --
OFF_LHST = 0                       # [256*16] lhsT fields per j
OFF_BIASF = OFF_LHST + NM * 16     # [256] BIG*(1-cm1*cm2)
OFF_BIASFN = OFF_BIASF + NM        # [256] -BIG*(1-cm1*cm2)
A_COLS = OFF_BIASFN + NM
# -- section B (stage-2, second DMA) --
OFF_IDENT = A_COLS                 # [128]
OFF_RECIPD = OFF_IDENT + P         # [8*16] 1/(cnt1*cnt2+eps)
OFF_NFAC = OFF_RECIPD + NI * NJ    # [8*16] 1-EPS/div
OFF_W1T = OFF_NFAC + NI * NJ       # [256]
OFF_W2T = OFF_W1T + 256            # [512]
OFF_B1 = OFF_W2T + 512             # [1]
OFF_B2 = OFF_B1 + 1                # [2]
OFF_EM = OFF_B2 + 2                # [16]
OFF_BIASE = OFF_EM + NJ            # [16]
OFF_BIASEN = OFF_BIASE + NJ        # [16]
OFF_RECIPE = OFF_BIASEN + NJ       # [1]
CPACK_COLS = OFF_RECIPE + 1

_CACHE = {}


def _build_program():
    import concourse.bass as bass
    import concourse.bacc as bacc
    import concourse.tile as tile
    import concourse.mybir as mybir

    f32 = mybir.dt.float32
    Alu = mybir.AluOpType
    Act = mybir.ActivationFunctionType
    AX = mybir.AxisListType

    nc = bacc.Bacc()

    d_in = nc.dram_tensor("d", [P, NM, D_C], f32, kind="ExternalInput")
    cpack_in = nc.dram_tensor("cpack", [P, CPACK_COLS], f32, kind="ExternalInput")
    out_t = nc.dram_tensor("out_t", [D_X, NI], f32, kind="ExternalOutput")

    with tile.TileContext(nc) as tc, ExitStack() as ctx:
        consts = ctx.enter_context(tc.tile_pool(name="consts", bufs=1))
        small = ctx.enter_context(tc.tile_pool(name="small", bufs=1))

        # ---------- constant loads: two DMAs (loop-critical part first) ----
        cpak = consts.tile([P, CPACK_COLS], f32)
        nc.sync.dma_start(out=cpak[:, 0:A_COLS], in_=cpack_in[:, 0:A_COLS])
        nc.sync.dma_start(
            out=cpak[:, A_COLS:CPACK_COLS], in_=cpack_in[:, A_COLS:CPACK_COLS]
        )
        lhsTs = cpak[:, OFF_LHST : OFF_LHST + NM * 16].rearrange(
            "p (j k) -> p j k", k=16
        )
        biasF = cpak[:, OFF_BIASF : OFF_BIASF + NM]
        biasFn = cpak[:, OFF_BIASFN : OFF_BIASFN + NM]
        ident = cpak[:, OFF_IDENT : OFF_IDENT + P]
        recipD = cpak[:, OFF_RECIPD : OFF_RECIPD + NI * NJ].rearrange(
            "p (I J) -> p I J", J=NJ
        )
        nfac = cpak[:, OFF_NFAC : OFF_NFAC + NI * NJ].rearrange(
            "p (I J) -> p I J", J=NJ
        )
        w1t_a = cpak[:, OFF_W1T : OFF_W1T + 128]
        w1t_b = cpak[:, OFF_W1T + 128 : OFF_W1T + 256]
        w2t_a = cpak[:, OFF_W2T : OFF_W2T + 256]
        w2t_b = cpak[:, OFF_W2T + 256 : OFF_W2T + 512]
        b1c = cpak[:, OFF_B1 : OFF_B1 + 1]
        b2c_a = cpak[:, OFF_B2 : OFF_B2 + 1]
        b2c_b = cpak[:, OFF_B2 + 1 : OFF_B2 + 2]
        emrep = cpak[:, OFF_EM : OFF_EM + NJ]
        biasE = cpak[:, OFF_BIASE : OFF_BIASE + NJ]
        biasEn = cpak[:, OFF_BIASEN : OFF_BIASEN + NJ]
        recipE = cpak[:, OFF_RECIPE : OFF_RECIPE + 1]

        # ---------- big J-loop ----------
        # umm[:, J, 0:64] = per-(i, J, c) biased max; [:, J, 64:128] = biased
        # min -- packed so ONE transpose per J lands ma-feats at partitions
        # 0:64 and mi-feats at 64:128.
        umm = consts.tile([P, NJ, P], f32)
        # z matrices in [feature, row=(I,J)] layout:
        #   rhs_z0: [0:64] m-feats, [64:128] mi-feats
        #   rhs_z1: [0:64] ma-feats, [64:128] std-feats
        rhs_z0 = small.tile([P, P], f32)
        rhs_z1 = small.tile([P, P], f32)
        Ssb = small.tile([16, NJ, P], f32)
        S2sb = small.tile([8, NJ, P], f32)

        with tc.tile_pool(name="psum_sums", bufs=1, space="PSUM") as psum_sums, \
             tc.tile_pool(name="psum_tr", bufs=2, space="PSUM") as psum_tr, \
             tc.tile_pool(name="loop", bufs=3) as loop_pool, \
             tc.tile_pool(name="btmp", bufs=3) as btmp_pool, \
             tc.tile_pool(name="gtmp", bufs=3) as gtmp_pool:
            S_ps = psum_sums.tile([16, NJ, P], f32)

            for J in range(NJ):
                # packed [d | d^2] tile
                dsq = loop_pool.tile([P, MA, P], f32, tag="dsq")
                nc.sync.dma_start(
                    out=dsq[:, :, 0:64], in_=d_in[:, J * MA : (J + 1) * MA, :]
                )
                nc.scalar.activation(
                    out=dsq[:, :, 64:128], in_=dsq[:, :, 0:64], func=Act.Square
                )

                # min path: bias-add on GpSimd for J < GMIN, else Vector
                menge = nc.gpsimd if J < GMIN else nc.vector
                mpool = gtmp_pool if J < GMIN else btmp_pool
                bt = mpool.tile([P, MA, D_C], f32, tag="bt")
                menge.tensor_tensor(
                    out=bt,
                    in0=dsq[:, :, 0:64],
                    in1=biasF[:, J * MA : (J + 1) * MA]
                    .unsqueeze(2)
                    .broadcast_to([P, MA, D_C]),
                    op=Alu.add,
                )
                nc.vector.tensor_reduce(
                    out=umm[:, J, 64:128],
                    in_=bt[:].rearrange("p a c -> p c a"),
                    axis=AX.X,
                    op=Alu.min,
                )
                # max path: bias-add on GpSimd for all J
                bt2 = gtmp_pool.tile([P, MA, D_C], f32, tag="bt2")
                nc.gpsimd.tensor_tensor(
                    out=bt2,
                    in0=dsq[:, :, 0:64],
                    in1=biasFn[:, J * MA : (J + 1) * MA]
                    .unsqueeze(2)
                    .broadcast_to([P, MA, D_C]),
                    op=Alu.add,
                )
                nc.vector.tensor_reduce(
                    out=umm[:, J, 0:64],
                    in_=bt2[:].rearrange("p a c -> p c a"),
                    axis=AX.X,
                    op=Alu.max,
                )

                # sums: ONE matmul per j, lhsT = [bind*cm1*cm2[j] | bind],
                # rhs = [d | d^2].  Rows 0:8 x cols 0:64 = Sm, rows 8:16 x
                # cols 0:64 = S1, rows 0:8 x cols 64:128 = S2.  (Masked
                # rows first so the S2 PSUM read starts at partition 0.)
                for jj in range(MA):
                    j = J * MA + jj
                    nc.tensor.matmul(
                        S_ps[:, J, :],
                        lhsT=lhsTs[:, j, :],
                        rhs=dsq[:, jj, :],
                        start=(jj == 0),
                        stop=(jj == MA - 1),
                    )

            # evacuate sums, duplicated into both 64-column halves so one
            # transpose serves lo and hi features.
            nc.scalar.copy(Ssb[:, :, 0:64], S_ps[0:16, :, 0:64])
            nc.scalar.copy(Ssb[:, :, 64:128], S_ps[0:16, :, 0:64])
            nc.scalar.copy(S2sb[:, :, 0:64], S_ps[0:8, :, 64:128])
            nc.scalar.copy(S2sb[:, :, 64:128], S_ps[0:8, :, 64:128])

            # min/max stage 2: one transpose per J into a PSUM ring of 4,
            # then ONE batched grouped reduce per 4-J round per feature-half.
            for Jr in range(0, NJ, 4):
                TP = psum_tr.tile([P, 4, P], f32, tag="tp")
                for k in range(4):
                    nc.tensor.transpose(
                        out=TP[:, k, :], in_=umm[:, Jr + k, :], identity=ident
                    )
                nc.vector.tensor_reduce(
                    out=rhs_z0[64:128, :]
                    .rearrange("p (I J) -> p J I", J=NJ)[:, Jr : Jr + 4, :],
                    in_=TP[64:128, :, :].rearrange("p r (I a) -> p r I a", a=MA),
                    axis=AX.X,
                    op=Alu.min,
                )
                nc.vector.tensor_reduce(
                    out=rhs_z1[0:64, :]
                    .rearrange("p (I J) -> p J I", J=NJ)[:, Jr : Jr + 4, :],
                    in_=TP[0:64, :, :].rearrange("p r (I a) -> p r I a", a=MA),
                    axis=AX.X,
                    op=Alu.max,
                )

        # ---------- stage 2: sums transposes to [feature, row] layout ----
        with tc.tile_pool(name="psum_ts", bufs=1, space="PSUM") as psum_ts, \
             tc.tile_pool(name="psum_e", bufs=1, space="PSUM") as psum_e, \
             tc.tile_pool(name="psum_o", bufs=1, space="PSUM") as psum_o:
            # sums: one transpose per J: [16, 128(dup)] -> [128, 16]
            # (columns = [S1T | SmT]); rows 0:64 serve m, 64:128 serve std.
            SST_ps = psum_ts.tile([P, 16, NJ], f32, tag="sst")
            S2T_ps = psum_ts.tile([P, NI, NJ], f32, tag="s2t")
            for J in range(NJ):
                nc.tensor.transpose(
                    out=SST_ps[:, :, J],
                    in_=Ssb[0:16, J, :],
                    identity=ident[0:16, 0:16],
                )
                nc.tensor.transpose(
                    out=S2T_ps[:, :, J],
                    in_=S2sb[0:8, J, :],
                    identity=ident[0:8, 0:8],
                )

            SST = small.tile([P, 16, NJ], f32)
            nc.scalar.copy(SST[:], SST_ps[:])
            S2T = small.tile([P, NI, NJ], f32)
            nc.scalar.copy(S2T[64:128], S2T_ps[64:128])
            S1T = SST[:, 8:16, :]
            SmT = SST[:, 0:8, :]

            # m = S1/div  (lo half -> m-features; hi half feeds std)
            mT = small.tile([P, NI, NJ], f32)
            nc.vector.tensor_tensor(out=mT[:], in0=S1T, in1=recipD, op=Alu.mult)
            nc.vector.tensor_copy(
                out=rhs_z0[0:64, :].rearrange("p (I J) -> p I J", J=NJ),
                in_=mT[0:64],
            )
            # std = S2/div - 2*m*(Sm/div) + m^2*nfac     (hi half only)
            A = small.tile([P, NI, NJ], f32)
            nc.vector.tensor_tensor(
                out=A[64:128], in0=S2T[64:128], in1=recipD[64:128], op=Alu.mult
            )
            Bq = small.tile([P, NI, NJ], f32)
            nc.vector.tensor_tensor(
                out=Bq[64:128], in0=SmT[64:128], in1=recipD[64:128], op=Alu.mult
            )
            nc.vector.tensor_tensor(
                out=Bq[64:128], in0=Bq[64:128], in1=mT[64:128], op=Alu.mult
            )
            nc.vector.tensor_scalar(
                Bq[64:128], Bq[64:128], -2.0, None, Alu.mult
            )  # -2*m*Sm/div
            nc.vector.tensor_tensor(
                out=A[64:128], in0=A[64:128], in1=Bq[64:128], op=Alu.add
            )
            Cq = small.tile([P, NI, NJ], f32)
            nc.vector.tensor_tensor(
                out=Cq[64:128], in0=mT[64:128], in1=mT[64:128], op=Alu.mult
            )
            nc.vector.tensor_tensor(
                out=Cq[64:128], in0=Cq[64:128], in1=nfac[64:128], op=Alu.mult
            )
            nc.vector.tensor_tensor(
                out=rhs_z1[64:128, :].rearrange("p (I J) -> p I J", J=NJ),
                in0=A[64:128],
                in1=Cq[64:128],
                op=Alu.add,
            )

            # ---------- E = z @ W1.T + b1 (dup channels on 128 parts) ----
            E_ps = psum_e.tile([P, P], f32)
            nc.tensor.matmul(
                E_ps[:], lhsT=w1t_a, rhs=rhs_z0[:], start=True, stop=False
            )
            nc.tensor.matmul(
                E_ps[:], lhsT=w1t_b, rhs=rhs_z1[:], start=False, stop=True
            )
            E_T = small.tile([P, P], f32)  # [128(dup chan), 128 rows=(I,J)]
            nc.scalar.activation(
                out=E_T[:], in_=E_ps[:], func=Act.Identity, bias=b1c, scale=1.0
            )

            # ---------- stage 2 of the net: masked stats over J ----------
            E_r = E_T[:].rearrange("p (I J) -> p I J", J=NJ)
            zE0 = small.tile([P, NI], f32)  # [0:64] mE, [64:128] miE
            zE1 = small.tile([P, NI], f32)  # [0:64] maE, [64:128] stdE

            # mE (all partitions; lo half is the m-feature, hi feeds stdE)
            mE = small.tile([P, NI], f32)
            nc.vector.tensor_reduce(out=mE[:], in_=E_r, axis=AX.X, op=Alu.add)
            nc.scalar.mul(mE[:], mE[:], recipE)
            nc.scalar.copy(zE0[0:64, :], mE[0:64, :])

            # miE on hi half (GpSimd)
            bE = small.tile([P, NI, NJ], f32)
            nc.gpsimd.tensor_tensor(
                out=bE[64:128],
                in0=E_r[64:128],
                in1=biasE[64:128].unsqueeze(1).broadcast_to([64, NI, NJ]),
                op=Alu.add,
            )
            nc.vector.tensor_reduce(
                out=zE0[64:128, :], in_=bE[64:128], axis=AX.X, op=Alu.min
            )
            # maE on lo half (Vector)
            bE2 = small.tile([P, NI, NJ], f32)
            nc.vector.tensor_tensor(
                out=bE2[0:64],
                in0=E_r[0:64],
                in1=biasEn[0:64].unsqueeze(1).broadcast_to([64, NI, NJ]),
                op=Alu.add,
            )
            nc.vector.tensor_reduce(
                out=zE1[0:64, :], in_=bE2[0:64], axis=AX.X, op=Alu.max
            )
            # stdE on hi half: sum(em*(E-mE)^2)/denom  (GpSimd)
            dev = small.tile([P, NI, NJ], f32)
            nc.gpsimd.tensor_tensor(
                out=dev[64:128],
                in0=E_r[64:128],
                in1=mE[64:128].unsqueeze(2).broadcast_to([64, NI, NJ]),
                op=Alu.subtract,
            )
            nc.gpsimd.tensor_tensor(
                out=dev[64:128], in0=dev[64:128], in1=dev[64:128], op=Alu.mult
            )
            nc.gpsimd.tensor_tensor(
                out=dev[64:128],
                in0=dev[64:128],
                in1=emrep[64:128].unsqueeze(1).broadcast_to([64, NI, NJ]),
                op=Alu.mult,
            )
            nc.vector.tensor_reduce(
                out=zE1[64:128, :], in_=dev[64:128], axis=AX.X, op=Alu.add
            )
            nc.scalar.mul(zE1[64:128, :], zE1[64:128, :], recipE[64:128])

            # ---------- out = zE @ W2.T + b2 ----------
            outa_ps = psum_o.tile([128, NI], f32)
            outb_ps = psum_o.tile([128, NI], f32)
            nc.tensor.matmul(
                outa_ps[:], lhsT=w2t_a[:, 0:128], rhs=zE0[:], start=True, stop=False
            )
            nc.tensor.matmul(
                outa_ps[:], lhsT=w2t_b[:, 0:128], rhs=zE1[:], start=False, stop=True
            )
            nc.tensor.matmul(
                outb_ps[:], lhsT=w2t_a[:, 128:256], rhs=zE0[:], start=True, stop=False
            )
            nc.tensor.matmul(
                outb_ps[:], lhsT=w2t_b[:, 128:256], rhs=zE1[:], start=False,
                stop=True,
            )
            outa = small.tile([128, NI], f32)
            nc.scalar.activation(
                out=outa[:], in_=outa_ps[:], func=Act.Identity, bias=b2c_a,
                scale=1.0,
            )
            outb = small.tile([128, NI], f32)
            nc.scalar.activation(
                out=outb[:], in_=outb_ps[:], func=Act.Identity, bias=b2c_b,
                scale=1.0,
            )
            nc.sync.dma_start(out=out_t[0:128, :], in_=outa[:])
            nc.sync.dma_start(out=out_t[128:256, :], in_=outb[:])

    nc.finalize()  # Bacc: runs compile() (wait splitting, reg alloc, ...)
    return nc


def _get_program():
    if "nc" not in _CACHE:
        _CACHE["nc"] = _build_program()
    return _CACHE["nc"]


def _make_in_maps(delta1, c_mask1, c_mask2, e_mask2, W1, b1, W2, b2):
    delta1 = np.asarray(delta1, dtype=np.float32)
    c_mask1 = np.asarray(c_mask1, dtype=np.float32)
    c_mask2 = np.asarray(c_mask2, dtype=np.float32)
    e_mask2 = np.asarray(e_mask2, dtype=np.float32)
    W1 = np.asarray(W1, dtype=np.float32)
    b1 = np.asarray(b1, dtype=np.float32)
    W2 = np.asarray(W2, dtype=np.float32)
    b2 = np.asarray(b2, dtype=np.float32)

    w1t = np.concatenate([W1.T, W1.T], axis=1)  # [256, 128] (dup out-chan)
    w2t = W2.T  # [256, 256]
    bindm = np.zeros((128, 8), dtype=np.float32)
    for i in range(128):
        bindm[i, i // 16] = 1.0
    identm = np.eye(128, dtype=np.float32)

    in_maps = []
    for k in range(8):
        b, ih = k // 2, k % 2
        cm1 = c_mask1[b, ih * 128 : (ih + 1) * 128, 0, 0]        # [128]
        cm2 = c_mask2[b, 0, :, 0]                                 # [256]
        em = e_mask2[b, 0, :, 0]                                  # [16]

        cp = np.zeros((128, CPACK_COLS), dtype=np.float32)
        # lhsT fields: [128, 256 j, 16]: cols 0:8 bind*cm1*cm2[j], 8:16 bind
        lhst = np.zeros((128, NM, 16), dtype=np.float32)
        lhst[:, :, 0:8] = (
            bindm[:, None, :] * cm1[:, None, None] * cm2[None, :, None]
        )
        lhst[:, :, 8:16] = bindm[:, None, :]
        cp[:, OFF_LHST : OFF_LHST + NM * 16] = lhst.reshape(128, NM * 16)
        t2 = cm1[:, None] * cm2[None, :]                          # [128, 256]
        cp[:, OFF_BIASF : OFF_BIASF + NM] = BIG * (1.0 - t2)
        cp[:, OFF_BIASFN : OFF_BIASFN + NM] = -BIG * (1.0 - t2)
        cp[:, OFF_IDENT : OFF_IDENT + P] = identm
        cnt1 = bindm.T @ cm1                                      # [8]
        cnt2 = cm2.reshape(NJ, MA).sum(axis=1)                    # [16]
        div = cnt1[:, None] * cnt2[None, :] + EPS                 # [8, 16]
        cp[:, OFF_RECIPD : OFF_RECIPD + NI * NJ] = (1.0 / div).reshape(-1)[None, :]
        cp[:, OFF_NFAC : OFF_NFAC + NI * NJ] = (1.0 - EPS / div).reshape(-1)[None, :]
        cp[:, OFF_W1T : OFF_W1T + 128] = w1t[0:128, :]
        cp[:, OFF_W1T + 128 : OFF_W1T + 256] = w1t[128:256, :]
        cp[:, OFF_W2T : OFF_W2T + 256] = w2t[0:128, :]
        cp[:, OFF_W2T + 256 : OFF_W2T + 512] = w2t[128:256, :]
        cp[:, OFF_B1] = np.concatenate([b1, b1])
        cp[:, OFF_B2] = b2[0:128]
        cp[:, OFF_B2 + 1] = b2[128:256]
        cp[:, OFF_EM : OFF_EM + NJ] = em[None, :]
        cp[:, OFF_BIASE : OFF_BIASE + NJ] = (BIG * (1.0 - em))[None, :]
        cp[:, OFF_BIASEN : OFF_BIASEN + NJ] = (-BIG * (1.0 - em))[None, :]
        cp[:, OFF_RECIPE] = 1.0 / em.sum()
        in_maps.append(
            dict(
                d=np.ascontiguousarray(delta1[b, ih * 128 : (ih + 1) * 128]),
                cpack=cp,
            )
        )
    return in_maps


def _assemble(results):
    out = np.empty((4, 16, 256), dtype=np.float32)
    for k in range(8):
        b, ih = k // 2, k % 2
        out[b, ih * 8 : (ih + 1) * 8, :] = results[k]["out_t"].T
    return out


def run(trace=False, **inputs):
    from concourse.bass_utils import run_bass_kernel_spmd

    nc = _get_program()
    in_maps = _make_in_maps(**inputs)
    res = run_bass_kernel_spmd(
        nc, in_maps, core_ids=list(range(8)), trace=trace
    )
    return _assemble(res.results), res


def kernel(**inputs):
    out, _ = run(trace=False, **inputs)
    return out
